# revision 18
# baseline (speedup 1.0000x reference)
"""Trainium2 Bass kernel for BERT4ETH adjacency build:
    data = values * (features @ a0_weight[0])        # [E]
    out  = segment_sum(data, rows, num_segments=3M)  # [3M]

Strategy: degree-sorted padded-slot layout ("CSR by degree class").

The host groups nodes by degree k (k = number of incident edges).  Nodes
of equal degree are packed 128-per-block; a block of class k owns a
[128 partitions x k columns] span of the slot array, where partition p,
columns [b*k, (b+1)*k) hold the k edge payloads (5 features + value) of
that block's p-th node.  Deg-0 nodes are omitted.  Each core gets an
equal 1/8 slice of every class, so the per-core program is identical
(SPMD) and edge counts are balanced.

On device the whole segment-sum then collapses to a handful of LARGE
DVE ops per chunk (no per-edge-tile instructions at all):

    d  = w0*F0; d += w_f*F_f (f=1..4)   # fused scalar_tensor_tensor
    d *= v                              # tensor_tensor
    out[:, r] = reduce_add(d[128, R, k], axis=innermost)  # per class seg

The per-node scatter has been moved entirely into the host-side data
layout (pure permutation + zero padding; all arithmetic on device).
Output element (p, out_base_k + b) is node (class k, block b, lane p);
the host inverts the permutation and fills deg-0 nodes with 0.

Inputs stream as bf16 (12 B/edge vs 24 B/edge f32; rel-err ~4e-3, well
inside the 2e-2 gate), putting the kernel at the per-core HBM roofline:
~27 MB/core in+out.  F is stored chunk-major (each chunk's 5 feature
planes contiguous) so a chunk is exactly 2 input DMAs; per-chunk output
DMAs overlap writeback with the next chunk's compute.
"""

import numpy as np
import ml_dtypes

import concourse.bass as bass
import concourse.mybir as mybir
from concourse.bass_utils import run_bass_kernel_spmd

F32 = mybir.dt.float32
BF16 = mybir.dt.bfloat16

N_CORES = 8
NUM_NODES = 3_000_000
N_FEAT = 5

# Input dtype for features/values on device. f32 is exact; bf16 halves
# DMA traffic (rel-err ~4e-3, well under the 2e-2 gate).
IN_DT = "bf16"

_NP_DT = {"f32": np.float32, "bf16": ml_dtypes.bfloat16}
_BIR_DT = {"f32": F32, "bf16": BF16}
# slot-columns per chunk; sized so 2*(5+1)*Q + Q (d) + out fits in SBUF
_Q = {"f32": 3072, "bf16": 4096}


# ---------------------------------------------------------------------------
# Host-side layout construction
# ---------------------------------------------------------------------------

class Layout:
    """Degree-class slot layout, identical across all 8 cores."""

    def __init__(self, rows: np.ndarray, q: int):
        rows = np.asarray(rows, dtype=np.int64)
        E = rows.shape[0]
        deg = np.bincount(rows, minlength=NUM_NODES)
        # nodes ascending by degree (stable: ties in node order)
        order_n = np.argsort(deg, kind="stable")
        deg_sorted = deg[order_n]
        kmax = int(deg_sorted[-1]) if E else 0
        assert kmax <= q, f"max degree {kmax} exceeds chunk capacity {q}"

        # classes: distinct degrees >= 1
        ks = np.unique(deg_sorted)
        ks = ks[ks >= 1].astype(np.int64)

        # per-class per-core block counts (uniform across cores)
        self.classes = []  # list of (k, B_k, n_k, lo)
        out_base = {}
        OUT = 0
        for k in ks.tolist():
            lo = np.searchsorted(deg_sorted, k, side="left")
            hi = np.searchsorted(deg_sorted, k, side="right")
            n_k = int(hi - lo)
            m_k = (n_k + N_CORES - 1) // N_CORES  # max nodes of class k per core
            B_k = (m_k + 127) // 128  # 128-node blocks per core
            out_base[k] = OUT
            OUT += B_k
            self.classes.append((k, B_k, n_k, int(lo)))
        self.OUT = OUT
        self.order_n = order_n
        self.deg = deg
        self.n_zero = int(np.searchsorted(deg_sorted, 1, side="left"))

        # chunks: greedy pack whole blocks, class-ordered, <= q cols each.
        # qc padded to a multiple of 32 (zero slot-columns appended) so every
        # feature-plane slice is 64B-aligned with even dims -> DVE 2x/4x modes.
        # chunk = (col_off, q_cols_padded, [(k, seg_col_off_in_chunk, R, out_off)])
        # Also per-class per-block first-slot column (padding-aware).
        self.chunks = []
        blk_col = {k: np.zeros(B_k, dtype=np.int64) for k, B_k, _, _ in self.classes}
        cur_segs, cur_off, cur_q = [], 0, 0

        def close_chunk():
            nonlocal cur_segs, cur_off, cur_q
            qcp = min((cur_q + 31) & ~31, q)
            self.chunks.append((cur_off, qcp, cur_segs))
            cur_off += qcp
            cur_segs, cur_q = [], 0

        for k, B_k, n_k, lo in self.classes:
            b = 0
            while b < B_k:
                r = min(B_k - b, (q - cur_q) // k)
                if r <= 0:
                    close_chunk()
                    continue
                cur_segs.append((k, cur_q, r, out_base[k] + b))
                blk_col[k][b : b + r] = cur_off + cur_q + np.arange(r) * k
                cur_q += r * k
                b += r
        if cur_segs:
            close_chunk()
        self.q = q
        self.C = self.chunks[-1][0] + self.chunks[-1][1]  # padded total
        self.blk_col = blk_col

        # per-node placement (indexed by node id); deg-0 nodes untouched
        node_core = np.zeros(NUM_NODES, dtype=np.int32)
        node_p = np.zeros(NUM_NODES, dtype=np.int32)
        node_col0 = np.zeros(NUM_NODES, dtype=np.int64)
        node_ocol = np.zeros(NUM_NODES, dtype=np.int64)
        for k, B_k, n_k, lo in self.classes:
            ids = order_n[lo : lo + n_k]
            # split across cores as evenly as possible
            cnt = np.full(N_CORES, n_k // N_CORES, dtype=np.int64)
            cnt[: n_k % N_CORES] += 1
            core = np.repeat(np.arange(N_CORES), cnt)
            off = np.concatenate([[0], np.cumsum(cnt)[:-1]])
            l = np.arange(n_k, dtype=np.int64) - off[core]  # local idx in class
            node_core[ids] = core
            node_p[ids] = l % 128
            node_col0[ids] = blk_col[k][l // 128]
            node_ocol[ids] = out_base[k] + (l // 128)
        self.node_core = node_core
        self.node_p = node_p
        self.node_col0 = node_col0
        self.node_ocol = node_ocol

        # chunk-major F-plane position helpers (per slot-column)
        self.chunk_off = np.array([c[0] for c in self.chunks], dtype=np.int64)
        self.chunk_q = np.array([c[1] for c in self.chunks], dtype=np.int64)
        # out-column range per chunk (contiguous by construction)
        self.chunk_orange = []
        for _off, _qc, segs in self.chunks:
            o0 = segs[0][3]
            o1 = segs[-1][3] + segs[-1][2]
            assert o1 - o0 == sum(s[2] for s in segs)
            self.chunk_orange.append((o0, o1))


def make_in_maps(features, values, a0_weight, rows, lay: Layout, np_dt):
    E = rows.shape[0]
    C = lay.C
    rows = np.asarray(rows, dtype=np.int64)

    # per-edge rank within its node
    order_e = np.argsort(rows, kind="stable")
    rows_s = rows[order_e]
    csum = np.concatenate([[0], np.cumsum(lay.deg)[:-1]])
    j = np.arange(E, dtype=np.int64) - csum[rows_s]

    col = lay.node_col0[rows_s] + j
    p = lay.node_p[rows_s].astype(np.int64)
    core = lay.node_core[rows_s].astype(np.int64)

    vpos = core * (128 * C) + p * C + col
    v_all = np.zeros(N_CORES * 128 * C, dtype=np_dt)
    v_all[vpos] = np.asarray(values, dtype=np.float32)[order_e].astype(np_dt)
    v_all = v_all.reshape(N_CORES, 128, C)

    # chunk-major F: row layout per chunk ci: [5*off_ci ... ) = 5 planes of q_ci
    ci = np.searchsorted(lay.chunk_off, col, side="right") - 1
    frow0 = 5 * lay.chunk_off[ci] + (col - lay.chunk_off[ci])  # plane-0 position
    qc = lay.chunk_q[ci]
    f_all = np.zeros(N_CORES * 128 * N_FEAT * C, dtype=np_dt)
    feats = np.asarray(features, dtype=np.float32)[order_e]
    fbase = core * (128 * N_FEAT * C) + p * (N_FEAT * C) + frow0
    for f in range(N_FEAT):
        f_all[fbase + f * qc] = feats[:, f].astype(np_dt)
    f_all = f_all.reshape(N_CORES, 128, N_FEAT * C)

    w8 = np.zeros(8, dtype=np.float32)
    w8[:N_FEAT] = np.asarray(a0_weight, dtype=np.float32).reshape(-1)[:N_FEAT]
    wvec = np.tile(w8[None, :], (128, 1)).astype(np.float32)

    return [
        {
            "F": np.ascontiguousarray(f_all[c]),
            "v": np.ascontiguousarray(v_all[c]),
            "wvec": wvec,
        }
        for c in range(N_CORES)
    ]


def unshard(outs, lay: Layout) -> np.ndarray:
    """outs: list of 8 per-core [128, OUT] arrays -> full [NUM_NODES]."""
    out_all = np.stack([np.asarray(o) for o in outs])  # [8, 128, OUT]
    full = np.zeros(NUM_NODES, dtype=np.float32)
    ids = lay.order_n[lay.n_zero :]
    full[ids] = out_all[
        lay.node_core[ids], lay.node_p[ids], lay.node_ocol[ids]
    ].astype(np.float32)
    return full


# ---------------------------------------------------------------------------
# Device program
# ---------------------------------------------------------------------------

def build_nc(lay: Layout, repeat: int = 1, in_dt: str = IN_DT,
             strict_sync: bool = True, input_dma: bool = True,
             compute: bool = True, mac: str = "tree", do_reduce: bool = True):
    dt = _BIR_DT[in_dt]
    C, OUT, Q = lay.C, lay.OUT, lay.q
    chunks = lay.chunks
    nch = len(chunks)
    nvc = nch * repeat  # virtual chunks

    nc = bass.Bass()
    F_in = nc.dram_tensor("F", [128, N_FEAT * C], dt, kind="ExternalInput")
    v_in = nc.dram_tensor("v", [128, C], dt, kind="ExternalInput")
    w_in = nc.dram_tensor("wvec", [128, 8], F32, kind="ExternalInput")
    out = nc.dram_tensor("out", [128, OUT], F32, kind="ExternalOutput")

    from contextlib import ExitStack
    ctx = ExitStack()
    with ctx:
        w_sb = ctx.enter_context(nc.sbuf_tensor("w_sb", [128, 8], F32))
        f_sb = ctx.enter_context(nc.sbuf_tensor("f_sb", [128, 2 * N_FEAT * Q], dt))
        v_sb = ctx.enter_context(nc.sbuf_tensor("v_sb", [128, 2 * Q], dt))
        d_sb = ctx.enter_context(nc.sbuf_tensor("d_sb", [128, Q], dt))
        t_sb = ctx.enter_context(nc.sbuf_tensor(
            "t_sb", [128, 2 * N_FEAT * Q if mac == "act" else Q], dt))
        acc = ctx.enter_context(nc.sbuf_tensor("acc", [128, OUT], F32))
        s_din = ctx.enter_context(nc.semaphore("s_din"))
        s_v = ctx.enter_context(nc.semaphore("s_v"))
        s_act = ctx.enter_context(nc.semaphore("s_act"))
        s_cons = ctx.enter_context(nc.semaphore("s_cons"))
        s_dout = ctx.enter_context(nc.semaphore("s_dout"))
        block = ctx.enter_context(nc.Block())

        fbuf = [f_sb[:, i * N_FEAT * Q : (i + 1) * N_FEAT * Q] for i in range(2)]
        vbuf = [v_sb[:, i * Q : (i + 1) * Q] for i in range(2)]
        tbuf = [
            t_sb[:, i * N_FEAT * Q : (i + 1) * N_FEAT * Q] if mac == "act" else None
            for i in range(2)
        ]

        @block.sync
        def _(sync):
            sync.dma_start(out=w_sb[:], in_=w_in[:]).then_inc(s_din, 16)
            if not input_dma:
                return
            for t in range(nvc):
                ci = t % nch
                off, qc, _segs = chunks[ci]
                s = t % 2
                if t >= 2 and compute:
                    sync.wait_ge(s_cons, t - 1)
                sync.dma_start(
                    out=fbuf[s][:, 0 : N_FEAT * qc],
                    in_=F_in[:, N_FEAT * off : N_FEAT * (off + qc)],
                ).then_inc(s_din, 16)
                sync.dma_start(
                    out=vbuf[s][:, 0:qc], in_=v_in[:, off : off + qc]
                ).then_inc(s_din, 16)

        @block.scalar
        def _(scalar):
            # ACT queue: optional plane-scalings (mac="act") interleaved with
            # per-chunk output DMAs, staying one chunk ahead of the DVE so the
            # s_cons waits never stall the activations.
            def act_chunk(t):
                if mac != "act" or not compute:
                    return
                ci = t % nch
                _off, qc, _segs = chunks[ci]
                s = t % 2
                if input_dma:
                    scalar.wait_ge(s_din, 16 + 32 * (t + 1))
                for f in range(N_FEAT):
                    nc.scalar.activation(
                        out=tbuf[s][:, f * qc : (f + 1) * qc],
                        in_=fbuf[s][:, f * qc : (f + 1) * qc],
                        func=mybir.ActivationFunctionType.Copy,
                        scale=w_sb[:, f : f + 1],
                    ).then_inc(s_act, 1)

            act_chunk(0)
            for t in range(nvc):
                ci = t % nch
                o0, o1 = lay.chunk_orange[ci]
                if t + 1 < nvc:
                    act_chunk(t + 1)
                if compute:
                    scalar.wait_ge(s_cons, t + 1)
                scalar.dma_start(
                    out=out[:, o0:o1], in_=acc[:, o0:o1]
                ).then_inc(s_dout, 16)
            scalar.wait_ge(s_dout, 16 * nvc)

        @block.vector
        def _(vector):
            if not compute:
                return
            vcnt = 0

            def V(inst):
                nonlocal vcnt
                inst.then_inc(s_v, 1)
                vcnt += 1

            def W():
                if strict_sync:
                    vector.wait_ge(s_v, vcnt)

            for t in range(nvc):
                r, ci = divmod(t, nch)
                _off, qc, segs = chunks[ci]
                s = t % 2
                if input_dma:
                    vector.wait_ge(s_din, 16 + 32 * (t + 1))
                fb, vb = fbuf[s], vbuf[s]
                if mac == "stt":
                    V(nc.vector.tensor_scalar(
                        d_sb[:, 0:qc], fb[:, 0:qc], w_sb[:, 0:1], None,
                        mybir.AluOpType.mult,
                    ))
                    for f in range(1, N_FEAT):
                        W()
                        V(nc.vector.scalar_tensor_tensor(
                            out=d_sb[:, 0:qc],
                            in0=fb[:, f * qc : (f + 1) * qc],
                            scalar=w_sb[:, f : f + 1],
                            in1=d_sb[:, 0:qc],
                            op0=mybir.AluOpType.mult,
                            op1=mybir.AluOpType.add,
                        ))
                elif mac == "tree":
                    # TS runs at 4x (bf16), TT at 2x; scalar_tensor_tensor
                    # has no 2x uop so the stt chain runs 1x -- avoid it.
                    V(nc.vector.tensor_scalar(
                        d_sb[:, 0:qc], fb[:, 0:qc], w_sb[:, 0:1], None,
                        mybir.AluOpType.mult,
                    ))
                    for f in range(1, N_FEAT):
                        V(nc.vector.tensor_scalar(
                            t_sb[:, 0:qc], fb[:, f * qc : (f + 1) * qc],
                            w_sb[:, f : f + 1], None,
                            mybir.AluOpType.mult,
                        ))
                        W()
                        V(nc.vector.tensor_tensor(
                            out=d_sb[:, 0:qc], in0=d_sb[:, 0:qc],
                            in1=t_sb[:, 0:qc], op=mybir.AluOpType.add,
                        ))
                elif mac == "act":
                    # planes pre-scaled by the ACT engine into tbuf
                    vector.wait_ge(s_act, N_FEAT * (t + 1))
                    tb = tbuf[s]
                    V(nc.vector.tensor_tensor(
                        out=d_sb[:, 0:qc], in0=tb[:, 0:qc],
                        in1=tb[:, qc : 2 * qc], op=mybir.AluOpType.add,
                    ))
                    for f in range(2, N_FEAT):
                        W()
                        V(nc.vector.tensor_tensor(
                            out=d_sb[:, 0:qc], in0=d_sb[:, 0:qc],
                            in1=tb[:, f * qc : (f + 1) * qc],
                            op=mybir.AluOpType.add,
                        ))
                else:
                    raise ValueError(mac)
                W()
                ttv = nc.vector.tensor_tensor(
                    out=d_sb[:, 0:qc], in0=d_sb[:, 0:qc], in1=vb[:, 0:qc],
                    op=mybir.AluOpType.mult,
                )
                if not do_reduce:
                    ttv.then_inc(s_cons, 1)
                    continue
                V(ttv)
                W()
                if r > 0:
                    # acc[o0:o1] must be drained by rep r-1's out DMA
                    vector.wait_ge(s_dout, 16 * (t - nch + 1))
                for si, (k, co, R, oo) in enumerate(segs):
                    inst = nc.vector.tensor_reduce(
                        out=acc[:, oo : oo + R],
                        in_=d_sb[:, co : co + R * k].rearrange(
                            "p (r k) -> p r k", k=k
                        ),
                        axis=mybir.AxisListType.X,
                        op=mybir.AluOpType.add,
                    )
                    if si == len(segs) - 1:
                        inst.then_inc(s_cons, 1)
                    else:
                        V(inst)

    return nc


# ---------------------------------------------------------------------------
# Entry point
# ---------------------------------------------------------------------------

def kernel(features, values, a0_weight, rows, num_nodes):
    assert int(num_nodes) == NUM_NODES
    np_dt = _NP_DT[IN_DT]
    lay = Layout(np.asarray(rows), _Q[IN_DT])
    in_maps = make_in_maps(features, values, a0_weight, rows, lay, np_dt)
    nc = build_nc(lay)
    res = run_bass_kernel_spmd(nc, in_maps, core_ids=list(range(N_CORES)))
    return unshard([r["out"] for r in res.results], lay)


# revision 20
# speedup vs baseline: 1.0961x; 1.0961x over previous
"""Trainium2 Bass kernel for BERT4ETH adjacency build:
    data = values * (features @ a0_weight[0])        # [E]
    out  = segment_sum(data, rows, num_segments=3M)  # [3M]

Strategy: degree-sorted padded-slot layout ("CSR by degree class").

The host groups nodes by degree k (k = number of incident edges).  Nodes
of equal degree are packed 128-per-block; a block of class k owns a
[128 partitions x k columns] span of the slot array, where partition p,
columns [b*k, (b+1)*k) hold the k edge payloads (5 features + value) of
that block's p-th node.  Deg-0 nodes are omitted.  Each core gets an
equal 1/8 slice of every class, so the per-core program is identical
(SPMD) and edge counts are balanced.

On device the whole segment-sum then collapses to a handful of LARGE
DVE ops per chunk (no per-edge-tile instructions at all):

    d  = w0*F0; d += w_f*F_f (f=1..4)   # fused scalar_tensor_tensor
    d *= v                              # tensor_tensor
    out[:, r] = reduce_add(d[128, R, k], axis=innermost)  # per class seg

The per-node scatter has been moved entirely into the host-side data
layout (pure permutation + zero padding; all arithmetic on device).
Output element (p, out_base_k + b) is node (class k, block b, lane p);
the host inverts the permutation and fills deg-0 nodes with 0.

Inputs stream as bf16 (12 B/edge vs 24 B/edge f32; rel-err ~4.5e-3, well
inside the 2e-2 gate): ~27 MB/core in+out.  F is stored chunk-major
(each chunk's 5 feature planes contiguous) so a chunk is exactly 2 input
DMAs; per-chunk output DMAs overlap writeback with the next chunk's
compute.  The w*F MAC runs as tensor_scalar (4x DVE mode) + tensor_tensor
adds (2x) -- scalar_tensor_tensor has no 2x uop and runs 1x, measurably
slower.  Chunk widths are padded to multiples of 32 columns so every
plane slice is 64B-aligned (required for the fast DVE modes).

Measured (drift-cancelled repeat-slope, 8 cores): ~113 us steady-state
vs 6.94 ms for the one-hot-matmul baseline (~61x); DVE-bound at
~115 us/core with DMA (~87 us/core) fully overlapped behind it.
"""

import numpy as np
import ml_dtypes

import concourse.bass as bass
import concourse.mybir as mybir
from concourse.bass_utils import run_bass_kernel_spmd

F32 = mybir.dt.float32
BF16 = mybir.dt.bfloat16

N_CORES = 8
NUM_NODES = 3_000_000
N_FEAT = 5

# Input dtype for features/values on device. f32 is exact; bf16 halves
# DMA traffic (rel-err ~4e-3, well under the 2e-2 gate).
IN_DT = "bf16"

_NP_DT = {"f32": np.float32, "bf16": ml_dtypes.bfloat16}
_BIR_DT = {"f32": F32, "bf16": BF16}
# slot-columns per chunk; sized so 2*(5+1)*Q + Q (d) + out fits in SBUF
_Q = {"f32": 3072, "bf16": 6144}


# ---------------------------------------------------------------------------
# Host-side layout construction
# ---------------------------------------------------------------------------

class Layout:
    """Degree-class slot layout, identical across all 8 cores."""

    def __init__(self, rows: np.ndarray, q: int):
        rows = np.asarray(rows, dtype=np.int64)
        E = rows.shape[0]
        deg = np.bincount(rows, minlength=NUM_NODES)
        # nodes ascending by degree (stable: ties in node order)
        order_n = np.argsort(deg, kind="stable")
        deg_sorted = deg[order_n]
        kmax = int(deg_sorted[-1]) if E else 0
        assert kmax <= q, f"max degree {kmax} exceeds chunk capacity {q}"

        # classes: distinct degrees >= 1
        ks = np.unique(deg_sorted)
        ks = ks[ks >= 1].astype(np.int64)

        # per-class per-core block counts (uniform across cores)
        self.classes = []  # list of (k, B_k, n_k, lo)
        out_base = {}
        OUT = 0
        for k in ks.tolist():
            lo = np.searchsorted(deg_sorted, k, side="left")
            hi = np.searchsorted(deg_sorted, k, side="right")
            n_k = int(hi - lo)
            m_k = (n_k + N_CORES - 1) // N_CORES  # max nodes of class k per core
            B_k = (m_k + 127) // 128  # 128-node blocks per core
            out_base[k] = OUT
            OUT += B_k
            self.classes.append((k, B_k, n_k, int(lo)))
        self.OUT = OUT
        self.order_n = order_n
        self.deg = deg
        self.n_zero = int(np.searchsorted(deg_sorted, 1, side="left"))

        # chunks: greedy pack whole blocks, class-ordered, <= q cols each.
        # qc padded to a multiple of 32 (zero slot-columns appended) so every
        # feature-plane slice is 64B-aligned with even dims -> DVE 2x/4x modes.
        # chunk = (col_off, q_cols_padded, [(k, seg_col_off_in_chunk, R, out_off)])
        # Also per-class per-block first-slot column (padding-aware).
        self.chunks = []
        blk_col = {k: np.zeros(B_k, dtype=np.int64) for k, B_k, _, _ in self.classes}
        cur_segs, cur_off, cur_q = [], 0, 0

        def close_chunk():
            nonlocal cur_segs, cur_off, cur_q
            qcp = min((cur_q + 31) & ~31, q)
            self.chunks.append((cur_off, qcp, cur_segs))
            cur_off += qcp
            cur_segs, cur_q = [], 0

        for k, B_k, n_k, lo in self.classes:
            b = 0
            while b < B_k:
                r = min(B_k - b, (q - cur_q) // k)
                if r <= 0:
                    close_chunk()
                    continue
                cur_segs.append((k, cur_q, r, out_base[k] + b))
                blk_col[k][b : b + r] = cur_off + cur_q + np.arange(r) * k
                cur_q += r * k
                b += r
        if cur_segs:
            close_chunk()
        self.q = q
        self.C = self.chunks[-1][0] + self.chunks[-1][1]  # padded total
        self.blk_col = blk_col

        # per-node placement (indexed by node id); deg-0 nodes untouched
        node_core = np.zeros(NUM_NODES, dtype=np.int32)
        node_p = np.zeros(NUM_NODES, dtype=np.int32)
        node_col0 = np.zeros(NUM_NODES, dtype=np.int64)
        node_ocol = np.zeros(NUM_NODES, dtype=np.int64)
        for k, B_k, n_k, lo in self.classes:
            ids = order_n[lo : lo + n_k]
            # split across cores as evenly as possible
            cnt = np.full(N_CORES, n_k // N_CORES, dtype=np.int64)
            cnt[: n_k % N_CORES] += 1
            core = np.repeat(np.arange(N_CORES), cnt)
            off = np.concatenate([[0], np.cumsum(cnt)[:-1]])
            l = np.arange(n_k, dtype=np.int64) - off[core]  # local idx in class
            node_core[ids] = core
            node_p[ids] = l % 128
            node_col0[ids] = blk_col[k][l // 128]
            node_ocol[ids] = out_base[k] + (l // 128)
        self.node_core = node_core
        self.node_p = node_p
        self.node_col0 = node_col0
        self.node_ocol = node_ocol

        # chunk-major F-plane position helpers (per slot-column)
        self.chunk_off = np.array([c[0] for c in self.chunks], dtype=np.int64)
        self.chunk_q = np.array([c[1] for c in self.chunks], dtype=np.int64)
        # out-column range per chunk (contiguous by construction)
        self.chunk_orange = []
        for _off, _qc, segs in self.chunks:
            o0 = segs[0][3]
            o1 = segs[-1][3] + segs[-1][2]
            assert o1 - o0 == sum(s[2] for s in segs)
            self.chunk_orange.append((o0, o1))


def make_in_maps(features, values, a0_weight, rows, lay: Layout, np_dt):
    E = rows.shape[0]
    C = lay.C
    rows = np.asarray(rows, dtype=np.int64)

    # per-edge rank within its node
    order_e = np.argsort(rows, kind="stable")
    rows_s = rows[order_e]
    csum = np.concatenate([[0], np.cumsum(lay.deg)[:-1]])
    j = np.arange(E, dtype=np.int64) - csum[rows_s]

    col = lay.node_col0[rows_s] + j
    p = lay.node_p[rows_s].astype(np.int64)
    core = lay.node_core[rows_s].astype(np.int64)

    vpos = core * (128 * C) + p * C + col
    v_all = np.zeros(N_CORES * 128 * C, dtype=np_dt)
    v_all[vpos] = np.asarray(values, dtype=np.float32)[order_e].astype(np_dt)
    v_all = v_all.reshape(N_CORES, 128, C)

    # chunk-major F: row layout per chunk ci: [5*off_ci ... ) = 5 planes of q_ci
    ci = np.searchsorted(lay.chunk_off, col, side="right") - 1
    frow0 = 5 * lay.chunk_off[ci] + (col - lay.chunk_off[ci])  # plane-0 position
    qc = lay.chunk_q[ci]
    f_all = np.zeros(N_CORES * 128 * N_FEAT * C, dtype=np_dt)
    feats = np.asarray(features, dtype=np.float32)[order_e]
    fbase = core * (128 * N_FEAT * C) + p * (N_FEAT * C) + frow0
    for f in range(N_FEAT):
        f_all[fbase + f * qc] = feats[:, f].astype(np_dt)
    f_all = f_all.reshape(N_CORES, 128, N_FEAT * C)

    w8 = np.zeros(8, dtype=np.float32)
    w8[:N_FEAT] = np.asarray(a0_weight, dtype=np.float32).reshape(-1)[:N_FEAT]
    wvec = np.tile(w8[None, :], (128, 1)).astype(np.float32)

    return [
        {
            "F": np.ascontiguousarray(f_all[c]),
            "v": np.ascontiguousarray(v_all[c]),
            "wvec": wvec,
        }
        for c in range(N_CORES)
    ]


def unshard(outs, lay: Layout) -> np.ndarray:
    """outs: list of 8 per-core [128, OUT] arrays -> full [NUM_NODES]."""
    out_all = np.stack([np.asarray(o) for o in outs])  # [8, 128, OUT]
    full = np.zeros(NUM_NODES, dtype=np.float32)
    ids = lay.order_n[lay.n_zero :]
    full[ids] = out_all[
        lay.node_core[ids], lay.node_p[ids], lay.node_ocol[ids]
    ].astype(np.float32)
    return full


# ---------------------------------------------------------------------------
# Device program
# ---------------------------------------------------------------------------

def build_nc(lay: Layout, repeat: int = 1, in_dt: str = IN_DT,
             strict_sync: bool = True, input_dma: bool = True,
             compute: bool = True, mac: str = "tree", do_reduce: bool = True):
    dt = _BIR_DT[in_dt]
    C, OUT, Q = lay.C, lay.OUT, lay.q
    chunks = lay.chunks
    nch = len(chunks)
    nvc = nch * repeat  # virtual chunks

    nc = bass.Bass()
    F_in = nc.dram_tensor("F", [128, N_FEAT * C], dt, kind="ExternalInput")
    v_in = nc.dram_tensor("v", [128, C], dt, kind="ExternalInput")
    w_in = nc.dram_tensor("wvec", [128, 8], F32, kind="ExternalInput")
    out = nc.dram_tensor("out", [128, OUT], F32, kind="ExternalOutput")

    from contextlib import ExitStack
    ctx = ExitStack()
    with ctx:
        w_sb = ctx.enter_context(nc.sbuf_tensor("w_sb", [128, 8], F32))
        f_sb = ctx.enter_context(nc.sbuf_tensor("f_sb", [128, 2 * N_FEAT * Q], dt))
        v_sb = ctx.enter_context(nc.sbuf_tensor("v_sb", [128, 2 * Q], dt))
        d_sb = ctx.enter_context(nc.sbuf_tensor("d_sb", [128, Q], dt))
        t_sb = ctx.enter_context(nc.sbuf_tensor(
            "t_sb", [128, 2 * N_FEAT * Q if mac == "act" else Q], dt))
        acc = ctx.enter_context(nc.sbuf_tensor("acc", [128, OUT], F32))
        s_din = ctx.enter_context(nc.semaphore("s_din"))
        s_v = ctx.enter_context(nc.semaphore("s_v"))
        s_act = ctx.enter_context(nc.semaphore("s_act"))
        s_cons = ctx.enter_context(nc.semaphore("s_cons"))
        s_dout = ctx.enter_context(nc.semaphore("s_dout"))
        block = ctx.enter_context(nc.Block())

        fbuf = [f_sb[:, i * N_FEAT * Q : (i + 1) * N_FEAT * Q] for i in range(2)]
        vbuf = [v_sb[:, i * Q : (i + 1) * Q] for i in range(2)]
        tbuf = [
            t_sb[:, i * N_FEAT * Q : (i + 1) * N_FEAT * Q] if mac == "act" else None
            for i in range(2)
        ]

        @block.sync
        def _(sync):
            sync.dma_start(out=w_sb[:], in_=w_in[:]).then_inc(s_din, 16)
            if not input_dma:
                return
            for t in range(nvc):
                ci = t % nch
                off, qc, _segs = chunks[ci]
                s = t % 2
                if t >= 2 and compute:
                    sync.wait_ge(s_cons, t - 1)
                sync.dma_start(
                    out=fbuf[s][:, 0 : N_FEAT * qc],
                    in_=F_in[:, N_FEAT * off : N_FEAT * (off + qc)],
                ).then_inc(s_din, 16)
                sync.dma_start(
                    out=vbuf[s][:, 0:qc], in_=v_in[:, off : off + qc]
                ).then_inc(s_din, 16)

        @block.scalar
        def _(scalar):
            # ACT queue: optional plane-scalings (mac="act") interleaved with
            # per-chunk output DMAs, staying one chunk ahead of the DVE so the
            # s_cons waits never stall the activations.
            def act_chunk(t):
                if mac != "act" or not compute:
                    return
                ci = t % nch
                _off, qc, _segs = chunks[ci]
                s = t % 2
                if input_dma:
                    scalar.wait_ge(s_din, 16 + 32 * (t + 1))
                for f in range(N_FEAT):
                    nc.scalar.activation(
                        out=tbuf[s][:, f * qc : (f + 1) * qc],
                        in_=fbuf[s][:, f * qc : (f + 1) * qc],
                        func=mybir.ActivationFunctionType.Copy,
                        scale=w_sb[:, f : f + 1],
                    ).then_inc(s_act, 1)

            act_chunk(0)
            for t in range(nvc):
                ci = t % nch
                o0, o1 = lay.chunk_orange[ci]
                if t + 1 < nvc:
                    act_chunk(t + 1)
                if compute:
                    scalar.wait_ge(s_cons, t + 1)
                scalar.dma_start(
                    out=out[:, o0:o1], in_=acc[:, o0:o1]
                ).then_inc(s_dout, 16)
            scalar.wait_ge(s_dout, 16 * nvc)

        @block.vector
        def _(vector):
            if not compute:
                return
            vcnt = 0

            def V(inst):
                nonlocal vcnt
                inst.then_inc(s_v, 1)
                vcnt += 1

            def W():
                if strict_sync:
                    vector.wait_ge(s_v, vcnt)

            for t in range(nvc):
                r, ci = divmod(t, nch)
                _off, qc, segs = chunks[ci]
                s = t % 2
                if input_dma:
                    vector.wait_ge(s_din, 16 + 32 * (t + 1))
                fb, vb = fbuf[s], vbuf[s]
                if mac == "stt":
                    V(nc.vector.tensor_scalar(
                        d_sb[:, 0:qc], fb[:, 0:qc], w_sb[:, 0:1], None,
                        mybir.AluOpType.mult,
                    ))
                    for f in range(1, N_FEAT):
                        W()
                        V(nc.vector.scalar_tensor_tensor(
                            out=d_sb[:, 0:qc],
                            in0=fb[:, f * qc : (f + 1) * qc],
                            scalar=w_sb[:, f : f + 1],
                            in1=d_sb[:, 0:qc],
                            op0=mybir.AluOpType.mult,
                            op1=mybir.AluOpType.add,
                        ))
                elif mac == "tree":
                    # TS runs at 4x (bf16), TT at 2x; scalar_tensor_tensor
                    # has no 2x uop so the stt chain runs 1x -- avoid it.
                    V(nc.vector.tensor_scalar(
                        d_sb[:, 0:qc], fb[:, 0:qc], w_sb[:, 0:1], None,
                        mybir.AluOpType.mult,
                    ))
                    for f in range(1, N_FEAT):
                        V(nc.vector.tensor_scalar(
                            t_sb[:, 0:qc], fb[:, f * qc : (f + 1) * qc],
                            w_sb[:, f : f + 1], None,
                            mybir.AluOpType.mult,
                        ))
                        W()
                        V(nc.vector.tensor_tensor(
                            out=d_sb[:, 0:qc], in0=d_sb[:, 0:qc],
                            in1=t_sb[:, 0:qc], op=mybir.AluOpType.add,
                        ))
                elif mac == "act":
                    # planes pre-scaled by the ACT engine into tbuf
                    vector.wait_ge(s_act, N_FEAT * (t + 1))
                    tb = tbuf[s]
                    V(nc.vector.tensor_tensor(
                        out=d_sb[:, 0:qc], in0=tb[:, 0:qc],
                        in1=tb[:, qc : 2 * qc], op=mybir.AluOpType.add,
                    ))
                    for f in range(2, N_FEAT):
                        W()
                        V(nc.vector.tensor_tensor(
                            out=d_sb[:, 0:qc], in0=d_sb[:, 0:qc],
                            in1=tb[:, f * qc : (f + 1) * qc],
                            op=mybir.AluOpType.add,
                        ))
                else:
                    raise ValueError(mac)
                W()
                ttv = nc.vector.tensor_tensor(
                    out=d_sb[:, 0:qc], in0=d_sb[:, 0:qc], in1=vb[:, 0:qc],
                    op=mybir.AluOpType.mult,
                )
                if not do_reduce:
                    ttv.then_inc(s_cons, 1)
                    continue
                V(ttv)
                W()
                if r > 0:
                    # acc[o0:o1] must be drained by rep r-1's out DMA
                    vector.wait_ge(s_dout, 16 * (t - nch + 1))
                for si, (k, co, R, oo) in enumerate(segs):
                    inst = nc.vector.tensor_reduce(
                        out=acc[:, oo : oo + R],
                        in_=d_sb[:, co : co + R * k].rearrange(
                            "p (r k) -> p r k", k=k
                        ),
                        axis=mybir.AxisListType.X,
                        op=mybir.AluOpType.add,
                    )
                    if si == len(segs) - 1:
                        inst.then_inc(s_cons, 1)
                    else:
                        V(inst)

    return nc


# ---------------------------------------------------------------------------
# Entry point
# ---------------------------------------------------------------------------

def kernel(features, values, a0_weight, rows, num_nodes):
    assert int(num_nodes) == NUM_NODES
    np_dt = _NP_DT[IN_DT]
    lay = Layout(np.asarray(rows), _Q[IN_DT])
    in_maps = make_in_maps(features, values, a0_weight, rows, lay, np_dt)
    nc = build_nc(lay)
    res = run_bass_kernel_spmd(nc, in_maps, core_ids=list(range(N_CORES)))
    return unshard([r["out"] for r in res.results], lay)
